# revision 13
# baseline (speedup 1.0000x reference)
"""Trainium2 Bass kernel for nn_Composer (mode-energy softmax), v2.

Math (reference, train path):
    mode_min = min_n phi[n, :]                       (global batch min, per mode)
    phic     = phi - mode_min + e                    (clamp at e is a no-op: min exact)
    L        = ln(phic)
    E[n,i]   = sum_j phic_i^G_ij * phic_j^(1-G_ij)   (G = symmetrized Gamma)
             = sum_j exp( G_ij*L_i + (1-G_ij)*L_j )   for j != i, diag via j==i col
    out      = (softmax(-t*E), log_softmax(-t*E)),  t = 1/sqrt(32)

v2 design (ACT is the bottleneck engine: the N*M^2 exp is ~109us/core floor):
  - ln in sample-major [128, 1024] at 8-chunk granularity (full-partition ACT,
    per-mode bias pre-added by DVE) instead of [32, F] mode-major.
  - Centered 1-pass f32r X matmul: X' = G*(L_i-c) + (1-G)*(L_j-c); exp bias=+c.
    Host-validated absmax 8.4e-4 vs the 2e-2 gate (same as old 3-pass).
  - 4-way row-tiled strip concurrency for the K=32 X matmuls (tile_position
    at all four 32-row groups), L replicated across strips by transposing a
    stride-0-repeated [128,(4,32)] copy of L (no SBUF->SBUF DMA replication).
  - -t folded into the f32r reduce stationary (z accumulated directly).
  - No max-subtraction in softmax (z in [-70,-15] is exp-safe in fp32);
    tail batched every 8 chunks: one exp, DVE sum, one ln, DVE reciprocal,
    DVE broadcast subtract/multiply.
  - PSUM plan (8 banks): lt4/ztps shared pool 2, xps 2x2, eps 2.

Two SPMD launches on 8 cores (data-parallel over the batch):
  launch 1: per-core per-mode min -> host combines 8x[32] -> mode_min
  launch 2: the main kernel above with bias32 = e - mode_min input
"""

import sys

sys.path.insert(0, "/opt/trn_rl_repo")

import numpy as np
import concourse.bass as bass
import concourse.tile as tile
from concourse import mybir
from concourse.bass_utils import run_bass_kernel_spmd
from concourse.masks import make_identity

N_CORES = 8
N_BATCH = 131072
M = 32
NS = N_BATCH // N_CORES  # 16384 samples per core
F = 512  # samples per chunk
NCHUNK = NS // F  # 32
GRP = 8  # chunks per ln/tail group
NGRP = NCHUNK // GRP  # 4
KREP = F // 128  # 4 sub-blocks of 128 samples per chunk
NSTRIP = 4  # row-strips used concurrently by the X matmuls
NPAIR = M // 4 // NSTRIP  # 2 outer iterations (8 iq tiles total)
E_CONST = float(np.e)
NEG_T = -1.0 / float(np.sqrt(M))
# Global centering constant for the exponent: L = ln(phic) in [1, ~2.53] for
# the N(0,1), N=131072 input regime; c = midpoint. Precision degrades only
# linearly and mildly if the actual range differs (validated: c=0 still ~2e-3).
C_CENTER = 1.7653

f32 = mybir.dt.float32
f32r = mybir.dt.float32r
AF = mybir.ActivationFunctionType
ALU = mybir.AluOpType


# ----------------------------------------------------------------------------
# helpers
# ----------------------------------------------------------------------------
def _split_sync_waits(nc, max_waits=1):
    """This container's walrus rejects >1 sync wait per instruction in some
    templates; hoist excess waits onto same-engine NOPs placed just before."""
    for fn in nc.m.functions:
        for bb in fn.blocks:
            new_list = []
            for ins in bb.instructions:
                si = getattr(ins, "sync_info", None)
                waits = list(si.on_wait) if si is not None else []
                if len(waits) > max_waits:
                    rest = waits[max_waits:]
                    del si.on_wait[max_waits:]
                    k = 0
                    while rest:
                        chunk, rest = rest[:max_waits], rest[max_waits:]
                        new_list.append(
                            mybir.InstNoOp(
                                name=f"{ins.name}-ws{k}",
                                engine=ins.engine,
                                ins=[],
                                outs=[],
                                sync_info=mybir.SyncInfo(on_wait=chunk, on_update=[]),
                            )
                        )
                        k += 1
                new_list.append(ins)
            bb.instructions[:] = new_list


def _view(ap, axes):
    """Rebuild an AP with explicit [stride, size] free axes (after partition)."""
    return bass.AP(tensor=ap.tensor, offset=ap.offset, ap=[ap.ap[0]] + axes)


def _bcast_inner(ap, rep):
    """[P, ...] -> [P, ..., rep] with stride-0 innermost axis."""
    return bass.AP(tensor=ap.tensor, offset=ap.offset, ap=list(ap.ap) + [[0, rep]])


# ----------------------------------------------------------------------------
# launch 1: per-core per-mode min
# ----------------------------------------------------------------------------
def build_min_nc(repeat=1):
    nc = bass.Bass()
    phi_d = nc.dram_tensor("phi", [NS, M], f32, kind="ExternalInput")
    out_d = nc.dram_tensor("pmin", [M, 1], f32, kind="ExternalOutput")

    NT = 4  # tiles of [128, 128 rows x 32 modes]
    ROWS = NS // NT // 128  # rows per partition per tile = 32

    with tile.TileContext(nc) as tc:
        with (
            tc.tile_pool(name="sb", bufs=2) as sb,
            tc.tile_pool(name="consts", bufs=2) as consts,
            tc.tile_pool(name="ps", bufs=2, space="PSUM") as ps,
        ):
            ident = consts.tile([128, 128], f32, tag="ident")
            make_identity(nc, ident)
            for _rep in range(repeat):
              mall = consts.tile([128, M], f32, tag="mall")
              for t in range(NT):
                  xt = sb.tile([128, ROWS * M], f32, tag="xt")
                  nc.sync.dma_start(
                      xt[:],
                      phi_d[t * (NS // NT) : (t + 1) * (NS // NT), :].rearrange(
                          "(p q) i -> p (q i)", p=128
                      ),
                  )
                  mt = sb.tile([128, M], f32, tag="mt")
                  # view free dim as (q, i) -> reduce over q (strided inner axis)
                  nc.vector.tensor_reduce(
                      mt[:],
                      xt[:].rearrange("p (q i) -> p i q", i=M),
                      axis=mybir.AxisListType.X,
                      op=ALU.min,
                  )
                  if t == 0:
                      nc.vector.tensor_copy(mall[:], mt[:])
                  else:
                      nc.vector.tensor_tensor(mall[:], mall[:], mt[:], op=ALU.min)
              pt = ps.tile([M, 128], f32, tag="pt")
              pm = consts.tile([M, 1], f32, tag="pm")
              nc.tensor.transpose(pt[:], mall[:], ident[:])
              nc.vector.tensor_reduce(pm[:], pt[:], axis=mybir.AxisListType.X, op=ALU.min)
              nc.sync.dma_start(out_d[:], pm[:])
    _split_sync_waits(nc)
    return nc


# ----------------------------------------------------------------------------
# launch 2: main kernel
# ----------------------------------------------------------------------------
def build_main_nc(repeat=1):
    nc = bass.Bass()
    phi_d = nc.dram_tensor("phi", [NS, M], f32, kind="ExternalInput")
    bias_d = nc.dram_tensor("bias32", [128, M], f32, kind="ExternalInput")
    a4_d = nc.dram_tensor("a4", [128, NPAIR, 128], f32, kind="ExternalInput")
    red_d = nc.dram_tensor("red", [128, M // 4, M], f32, kind="ExternalInput")
    alphas_d = nc.dram_tensor("alphas", [NS, M], f32, kind="ExternalOutput")
    logits_d = nc.dram_tensor("logits", [NS, M], f32, kind="ExternalOutput")

    FG = F * GRP  # samples per group = 4096
    LNF = KREP * GRP * M  # ln tile free size = 1024

    with tile.TileContext(nc) as tc:
        with (
            tc.tile_pool(name="consts", bufs=1) as consts,
            tc.tile_pool(name="inb", bufs=2) as inb,
            tc.tile_pool(name="lab", bufs=2) as lab,
            tc.tile_pool(name="la4b", bufs=2) as la4b,
            tc.tile_pool(name="dlb", bufs=3) as dlb,
            tc.tile_pool(name="ttb", bufs=3) as ttb,
            tc.tile_pool(name="zsb", bufs=2) as zsb,
            tc.tile_pool(name="zbigb", bufs=2) as zbigb,
            tc.tile_pool(name="tailb", bufs=2) as tailb,
            tc.tile_pool(name="outb", bufs=2) as outb,
            tc.tile_pool(name="ltp", bufs=2, space="PSUM") as ltp,
            tc.tile_pool(name="xps", bufs=2, space="PSUM") as xpsp,
            tc.tile_pool(name="eps", bufs=1, space="PSUM") as epsp,
            tc.tile_pool(name="ztp", bufs=1, space="PSUM") as ztpp,
        ):
            ident = consts.tile([128, 128], f32)
            make_identity(nc, ident)
            cvec = consts.tile([128, 1], f32)
            nc.vector.memset(cvec[:], C_CENTER)
            bias_s = consts.tile([128, M], f32)
            a4_s = consts.tile([128, NPAIR, 128], f32)
            red_s = consts.tile([128, M // 4, M], f32)
            nc.sync.dma_start(bias_s[:], bias_d[:])
            nc.sync.dma_start(a4_s[:], a4_d[:])
            nc.sync.dma_start(red_s[:], red_d[:])
            # round constants into f32r tiles (device rounding = the producer
            # the BIR verifier requires for f32r matmul operands)
            a4_r = consts.tile([128, NPAIR, 128], f32r)
            red_r = consts.tile([128, M // 4, M], f32r)
            nc.vector.tensor_copy(a4_r[:], a4_s[:])
            nc.vector.tensor_copy(red_r[:], red_s[:])

            for _rep in range(repeat):
              for g in range(NGRP):
                r0 = g * FG
                # ---- load [FG, 32] as [128, (GRP*KREP, 32)]
                phia = inb.tile([128, GRP * KREP, M], f32, tag="phia")
                nc.sync.dma_start(
                    phia[:],
                    phi_d[r0 : r0 + FG, :].rearrange("(k p) i -> p k i", p=128),
                )
                # ---- phib = phi + (e - mode_min); ln in sample-major
                phib = inb.tile([128, GRP * KREP, M], f32, tag="phib")
                nc.vector.tensor_tensor(
                    phib[:],
                    phia[:],
                    _view(bias_s[:], [[0, GRP * KREP], [1, M]]),
                    op=ALU.add,
                )
                la = lab.tile([128, GRP * KREP, M], f32, tag="la")
                nc.scalar.activation(la[:], phib[:], AF.Ln)

                for cc in range(GRP):
                    # ---- la4[p, k, s, m] = la[p, cc*KREP + k, m] (stride-0
                    # s-axis): replicate L across the 4 row strips
                    la4 = la4b.tile([128, KREP, NSTRIP, M], f32, tag="la4")
                    src = bass.AP(
                        tensor=la[:].tensor,
                        offset=la[:].offset + cc * KREP * M,
                        ap=[la[:].ap[0], [M, KREP], [0, NSTRIP], [1, M]],
                    )
                    nc.vector.tensor_copy(la4[:], src)

                    # ---- transpose to mode-major, 4-strip replicated:
                    # lt4[(s,m), 128k + n] = L[n, m]
                    lt4 = ltp.tile([128, F], f32, tag="lt4")
                    for k in range(KREP):
                        nc.tensor.transpose(
                            lt4[:, k * 128 : (k + 1) * 128],
                            la4[:, k, :, :],
                            ident[:],
                        )
                    # ---- center + f32r round (PSUM -> SBUF)
                    dl4 = dlb.tile([128, F], f32r, tag="dl4")
                    nc.vector.tensor_scalar(
                        dl4[:], lt4[:], C_CENTER, None, op0=ALU.subtract
                    )

                    # ---- X matmuls (1-pass f32r, 4 concurrent row strips),
                    # exp with bias=+c, then z-accumulating reduce matmuls
                    eps = epsp.tile([M, F], f32, tag="eps")
                    for t in range(NPAIR):
                        xa = xpsp.tile([128, 2 * F], f32, tag="xps")
                        xb = xpsp.tile([128, 2 * F], f32, tag="xps")
                        for s in range(NSTRIP):
                            dst = (xa if s < 2 else xb)[:, (s % 2) * F : (s % 2 + 1) * F]
                            nc.tensor.matmul(
                                dst,
                                a4_r[32 * s : 32 * s + 32, t, :],
                                dl4[32 * s : 32 * s + 32, :],
                                start=True,
                                stop=True,
                                tile_position=(32 * s, 0),
                                skip_group_check=True,
                            )
                        tta = ttb.tile([128, 2 * F], f32r, tag="tt")
                        ttbt = ttb.tile([128, 2 * F], f32r, tag="tt")
                        nc.scalar.activation(tta[:], xa[:], AF.Exp, bias=cvec[:])
                        nc.scalar.activation(ttbt[:], xb[:], AF.Exp, bias=cvec[:])
                        for s in range(NSTRIP):
                            iq = NSTRIP * t + s
                            src_tt = (tta if s < 2 else ttbt)[
                                :, (s % 2) * F : (s % 2 + 1) * F
                            ]
                            nc.tensor.matmul(
                                eps[:],
                                red_r[:, iq, :],
                                src_tt,
                                start=(t == 0 and s == 0),
                                stop=(t == NPAIR - 1 and s == NSTRIP - 1),
                                skip_group_check=True,
                            )

                    # ---- z (already -t*E) to SBUF, transpose to sample-major
                    zs = zsb.tile([M, F], f32, tag="zs")
                    nc.vector.tensor_copy(zs[:], eps[:])
                    ztps = ztpp.tile([128, KREP * M], f32, tag="ztps")
                    for k in range(KREP):
                        nc.tensor.transpose(
                            ztps[:, k * M : (k + 1) * M],
                            zs[:, k * 128 : (k + 1) * 128],
                            ident[:M, :M],
                        )
                    if cc == 0:
                        zbig = zbigb.tile([128, GRP, KREP, M], f32, tag="zbig")
                    nc.vector.tensor_copy(
                        zbig[:, cc, :, :].rearrange("p k m -> p (k m)"), ztps[:]
                    )

                # ---- tail for the group: log-softmax with exact max-subtract
                # (keeps dominant-mode logits relatively accurate)
                zv = zbig[:].rearrange("p c k m -> p (c k) m")
                m4 = tailb.tile([128, GRP * KREP], f32, tag="m4")
                nc.vector.tensor_reduce(
                    m4[:], zv, axis=mybir.AxisListType.X, op=ALU.max
                )
                x2 = tailb.tile([128, GRP * KREP, M], f32, tag="x2")
                nc.vector.tensor_tensor(
                    x2[:], zv, _bcast_inner(m4[:], M), op=ALU.subtract
                )
                pz = tailb.tile([128, GRP * KREP, M], f32, tag="pz")
                nc.scalar.activation(
                    pz[:].rearrange("p k m -> p (k m)"),
                    x2[:].rearrange("p k m -> p (k m)"),
                    AF.Exp,
                )
                ssum = tailb.tile([128, GRP * KREP], f32, tag="ssum")
                nc.vector.tensor_reduce(
                    ssum[:], pz[:], axis=mybir.AxisListType.X, op=ALU.add
                )
                lns = tailb.tile([128, GRP * KREP], f32, tag="lns")
                nc.scalar.activation(lns[:], ssum[:], AF.Ln)
                rinv = tailb.tile([128, GRP * KREP], f32, tag="rinv")
                nc.vector.reciprocal(rinv[:], ssum[:])
                logit_s = outb.tile([128, GRP * KREP, M], f32, tag="logit_s")
                nc.vector.tensor_tensor(
                    logit_s[:], x2[:], _bcast_inner(lns[:], M), op=ALU.subtract
                )
                alpha_s = outb.tile([128, GRP * KREP, M], f32, tag="alpha_s")
                nc.vector.tensor_tensor(
                    alpha_s[:], pz[:], _bcast_inner(rinv[:], M), op=ALU.mult
                )
                nc.sync.dma_start(
                    logits_d[r0 : r0 + FG, :].rearrange("(k p) i -> p k i", p=128),
                    logit_s[:],
                )
                nc.sync.dma_start(
                    alphas_d[r0 : r0 + FG, :].rearrange("(k p) i -> p k i", p=128),
                    alpha_s[:],
                )
    _split_sync_waits(nc)
    return nc


# ----------------------------------------------------------------------------
# host-side constants from Gamma / w
# ----------------------------------------------------------------------------
def build_stationaries(Gamma, w):
    idx = np.arange(M)
    G = np.where(idx[:, None] < idx[None, :], Gamma, Gamma.T).astype(np.float64)

    # X stationaries: A_iq[k, 32g + j] = G[i,j]*[k==i] + (1-G[i,j])*[k==j],
    # i = 4*iq + g, packed NSTRIP iqs per 128-partition tile (strip s: iq=4t+s)
    A = np.zeros((M // 4, M, 128), dtype=np.float64)
    for iq in range(M // 4):
        for gg in range(4):
            i = 4 * iq + gg
            for j in range(M):
                col = 32 * gg + j
                A[iq, i, col] += G[i, j]
                A[iq, j, col] += 1.0 - G[i, j]
    A = A.astype(np.float32)

    a4 = np.zeros((128, NPAIR, 128), dtype=np.float32)
    for t in range(NPAIR):
        for s in range(NSTRIP):
            iq = NSTRIP * t + s
            a4[32 * s : 32 * s + 32, t, :] = A[iq]

    # reducer: z_m = -t * sum_iq sum_k S_iq[k, m] T_iq[k, n]
    # S_iq[k=(g,j), m] = [m == 4iq + g]  (+ w_m at the diagonal entry j==i)
    red = np.zeros((128, M // 4, M), dtype=np.float32)
    for iq in range(M // 4):
        S = np.zeros((128, M), dtype=np.float64)
        for gg in range(4):
            i = 4 * iq + gg
            S[32 * gg : 32 * gg + 32, i] += 1.0
            S[32 * gg + i, i] += float(w[i])
        red[:, iq, :] = (NEG_T * S).astype(np.float32)
    return a4, red


def main_in_maps(shards, mode_min, Gamma, w):
    """Input maps for build_main_nc given per-core phi shards + host params."""
    bias32 = np.tile((E_CONST - mode_min).reshape(1, M), (128, 1)).astype(np.float32)
    a4, red = build_stationaries(Gamma, w)
    return [
        {"phi": s, "bias32": bias32, "a4": a4, "red": red}
        for s in shards
    ]


_NC_CACHE = {}


def _get_ncs():
    if "min" not in _NC_CACHE:
        _NC_CACHE["min"] = build_min_nc()
        _NC_CACHE["main"] = build_main_nc()
    return _NC_CACHE["min"], _NC_CACHE["main"]


def kernel(phi, Gamma, w):
    phi = np.ascontiguousarray(np.asarray(phi), dtype=np.float32)
    Gamma = np.asarray(Gamma, dtype=np.float32)
    w = np.asarray(w, dtype=np.float32)
    assert phi.shape == (N_BATCH, M)

    nc_min, nc_main = _get_ncs()
    core_ids = list(range(N_CORES))
    shards = [phi[c * NS : (c + 1) * NS] for c in range(N_CORES)]

    # launch 1: global per-mode min
    res1 = run_bass_kernel_spmd(
        nc_min, [{"phi": s} for s in shards], core_ids=core_ids
    ).results
    mode_min = np.min(
        np.stack([r["pmin"][:, 0] for r in res1], axis=0), axis=0
    ).astype(np.float32)

    # launch 2: main kernel
    in_maps = main_in_maps(shards, mode_min, Gamma, w)
    res2 = run_bass_kernel_spmd(nc_main, in_maps, core_ids=core_ids).results
    alphas = np.concatenate([r["alphas"] for r in res2], axis=0)
    logits = np.concatenate([r["logits"] for r in res2], axis=0)
    return alphas, logits
